# revision 1
# baseline (speedup 1.0000x reference)
"""BatchBlobLoss Trainium2 kernel (8-core SPMD).

Reference computation:
  p = softmax(predictions, axis=1)[:, 1:]          # foreground class probs
  per (b, c): segment-sum of p keyed by instance id t = targets[b, c]
  soft-dice per (b, c, instance), masked mean -> scalar.

Device strategy (per core; cores k = 0..7 get batch b = k//4 and
D-slices 16*(k%4) .. +16):
  The 33-bin segment sum is computed with one fused reduce-op per bin:
    x = t + p  (p in (0,1) strictly, so floor(x) = t)
    ACT (scalar engine):  G_m  = sum relu(x - m)       = B_m + sum_{i>m} N_{>=i}
    ACT (Sign):           S_m  = sum sign(x - m)       = 2*N_{>=m} - n
    DVE (is_ge + accum):  N_{>=m} = sum [x >= m]
  where B_m = sum_{t >= m} p. Host (float64) recovers
    P_m = B_m - B_{m+1}  (per-instance prob sums) and C_m = N_{>=m} - N_{>=m+1}
  and evaluates the tiny dice formula. Per-partition accumulator strips
  [128, n_cols] are DMA'd out and reduced on host.
"""
import numpy as np
from contextlib import ExitStack

import concourse.bass as bass
import concourse.tile as tile
from concourse import bacc, mybir
from concourse import bass_utils
from concourse.bass_interp import get_hw_module

# problem shape (hardcoded per contest rules)
B, C, D, H, W = 2, 3, 64, 256, 256
M = 32
EPS = 1e-5
N_CORES = 8
CORES_PER_BATCH = 4
D_SH = D // CORES_PER_BATCH      # 16 depth slices per core
P = 128
NVOX = D_SH * H * W              # 1,048,576 voxels per core per channel
F = NVOX // P                    # 8192
CHUNK = 4096
NCHUNK = F // CHUNK              # 2
NBINS = 33                       # ids 0..32

# engine split for the 65 binning passes per channel (full 8192-rows)
VAL_SPLIT = list(range(0, 6))     # value bins binned per-chunk (overlap prep)
VAL_ACT = list(range(6, NBINS))   # value bins via ACT Relu -> G_m (full row)
VAL_DVE = []                      # value bins via DVE (sub,max) -> G_m
CNT_SPLIT_DVE = [1, 2, 3]         # count bins per-chunk on raw t (DVE is_ge)
CNT_ACT = [30, 31, 32]            # count bins per-chunk on raw t (ACT Sign)
CNT_HALF = 4                      # chunk0 on DVE, chunk1 on ACT Sign
CNT_DVE = list(range(5, 30))      # count bins via DVE is_ge on x (full row)

COLS_PER_SET = 65                 # 33 value + 32 count columns
N_VSPLIT_COLS = 2 * len(VAL_SPLIT)   # second-chunk cols for split value bins
N_CSPLIT = CNT_SPLIT_DVE + CNT_ACT + [CNT_HALF]  # per-chunk count bins
N_CSPLIT_COLS = 2 * len(N_CSPLIT)    # second-chunk cols for split count bins
N_COLS = 2 * COLS_PER_SET + N_VSPLIT_COLS + N_CSPLIT_COLS

F32 = mybir.dt.float32
BF16 = mybir.dt.bfloat16
I32 = mybir.dt.int32


def _val_col(ch, m):
    return ch * COLS_PER_SET + m


def _cnt_col(ch, m):
    return ch * COLS_PER_SET + NBINS + (m - 1)


def _split_col(ch, i):
    # chunk-1 partial for VAL_SPLIT[i]; chunk-0 partial lives in _val_col
    return 2 * COLS_PER_SET + ch * len(VAL_SPLIT) + i


def _csplit_col(ch, i):
    # chunk-1 partial for N_CSPLIT[i]; chunk-0 partial lives in _cnt_col
    return (2 * COLS_PER_SET + N_VSPLIT_COLS + ch * len(N_CSPLIT) + i)


def build_nc(scopes=False):
    AluOp = mybir.AluOpType
    ACT = mybir.ActivationFunctionType

    import contextlib

    def sc(nc, name):
        return nc.named_scope(name) if scopes else contextlib.nullcontext()

    nc = bacc.Bacc("TRN2", target_bir_lowering=False, debug=False,
                   num_devices=N_CORES)
    pred = nc.dram_tensor("pred", [C, P, F], F32, kind="ExternalInput").ap()
    targ = nc.dram_tensor("targ", [2, P, F], I32, kind="ExternalInput").ap()
    out = nc.dram_tensor("out", [P, N_COLS], F32, kind="ExternalOutput").ap()
    out_a = nc.dram_tensor("out_a", [P, N_COLS], F32,
                           kind="ExternalOutput").ap()

    with tile.TileContext(nc) as tc:
        with ExitStack() as ctx:
            pool = ctx.enter_context(tc.tile_pool(name="main", bufs=1))

            # bias strip: column m holds -m (f32), for ACT bias
            bias_i = pool.tile([P, NBINS], I32, tag="bias_i")
            nc.gpsimd.iota(bias_i[:], [[1, NBINS]], channel_multiplier=0)
            bias_f = pool.tile([P, NBINS], F32, tag="bias_f")
            nc.vector.tensor_scalar(bias_f[:], bias_i[:], -1.0, None, AluOp.mult)
            # half-shifted bias for Sign on raw integer t: sign(t - m + 0.5)
            bias_h = pool.tile([P, NBINS], F32, tag="bias_h")
            nc.vector.tensor_scalar(bias_h[:], bias_f[:], 0.5, None, AluOp.add)

            strip = pool.tile([P, N_COLS], F32, tag="strip")
            strip_a = pool.tile([P, N_COLS], F32, tag="strip_a")
            nc.gpsimd.memset(strip[:], 0.0)
            nc.gpsimd.memset(strip_a[:], 0.0)

            ones = pool.tile([P, 1], F32, tag="ones")
            nc.gpsimd.memset(ones[:], 1.0)

            trash_a = pool.tile([P, F], BF16, tag="trash_a")
            trash_d = pool.tile([P, F], BF16, tag="trash_d")

            xp1 = pool.tile([P, F], F32, tag="xp1")
            xp2 = pool.tile([P, F], F32, tag="xp2")

            for chunk in range(NCHUNK):
                sl = bass.ts(chunk, CHUNK)
                # load logits + targets for this chunk
                x0 = pool.tile([P, CHUNK], F32, tag="x0", bufs=1)
                x1 = pool.tile([P, CHUNK], F32, tag="x1", bufs=1)
                x2 = pool.tile([P, CHUNK], F32, tag="x2", bufs=1)
                t1 = pool.tile([P, CHUNK], I32, tag="t1", bufs=1)
                t2 = pool.tile([P, CHUNK], I32, tag="t2", bufs=1)
                scr = pool.tile([P, CHUNK], F32, tag="scr", bufs=1)
                nc.sync.dma_start(x0[:], pred[0, :, sl])
                nc.sync.dma_start(x1[:], pred[1, :, sl])
                nc.sync.dma_start(x2[:], pred[2, :, sl])
                nc.sync.dma_start(t1[:], targ[0, :, sl])
                nc.sync.dma_start(t2[:], targ[1, :, sl])

                with sc(nc, f"prep_exp{chunk}"):
                    # in-place exp: x_c <- e_c
                    nc.scalar.activation(x0[:], x0[:], ACT.Exp)
                    nc.scalar.activation(x1[:], x1[:], ACT.Exp)
                    nc.scalar.activation(x2[:], x2[:], ACT.Exp)
                with sc(nc, f"prep_dve{chunk}"):
                    # s = e0 + e1 + e2 into scr
                    nc.vector.tensor_tensor(scr[:], x0[:], x1[:], AluOp.add)
                    nc.vector.tensor_tensor(scr[:], scr[:], x2[:], AluOp.add)
                    # r = 1/s into x0 (e0 dead)
                    nc.vector.reciprocal_approx_fast(x0[:], scr[:])
                    # p1, p2 in place
                    nc.vector.tensor_tensor(x1[:], x1[:], x0[:], AluOp.mult)
                    nc.vector.tensor_tensor(x2[:], x2[:], x0[:], AluOp.mult)
                    # packed x = t + p into the full-row tiles
                    nc.vector.scalar_tensor_tensor(
                        xp1[:, sl], t1[:], 0.0, x1[:], AluOp.add, AluOp.add)
                    nc.vector.scalar_tensor_tensor(
                        xp2[:, sl], t2[:], 0.0, x2[:], AluOp.add, AluOp.add)

                # count bins on the raw int32 targets -- these only need the
                # t DMA, so they fill the engine-idle windows before/during
                # softmax prep
                with sc(nc, f"cnt_t{chunk}"):
                    for ch, tc_ in ((0, t1), (1, t2)):
                        for i, m in enumerate(CNT_SPLIT_DVE):
                            col = (_cnt_col(ch, m) if chunk == 0
                                   else _csplit_col(ch, i))
                            nc.vector.scalar_tensor_tensor(
                                trash_d[:, sl], tc_[:], float(m),
                                ones[:].to_broadcast((P, CHUNK)),
                                AluOp.is_ge, AluOp.mult,
                                accum_out=strip[:, col:col + 1])
                        for j, m in enumerate(CNT_ACT):
                            i = len(CNT_SPLIT_DVE) + j
                            col = (_cnt_col(ch, m) if chunk == 0
                                   else _csplit_col(ch, i))
                            nc.scalar.activation(
                                trash_a[:, sl], tc_[:], ACT.Sign,
                                bias=bias_h[:, m:m + 1], scale=1.0,
                                accum_out=strip_a[:, col:col + 1])

                # split value bins: bin this chunk's halves now so ACT has
                # work while the other chunk is being prepped
                with sc(nc, f"bin_split{chunk}"):
                    for ch, xpc in ((0, xp1), (1, xp2)):
                        for i, m in enumerate(VAL_SPLIT):
                            col = (_val_col(ch, m) if chunk == 0
                                   else _split_col(ch, i))
                            nc.scalar.activation(
                                trash_a[:, sl], xpc[:, sl], ACT.Relu,
                                bias=bias_f[:, m:m + 1], scale=1.0,
                                accum_out=strip_a[:, col:col + 1])

            # binning over the full [P, F] packed rows
            # ACT: all Relu ops first (both channels), then all Sign ops --
            # minimizes activation-table switches.
            with sc(nc, "bin_act_v"):
                for ch, xc in ((0, xp1), (1, xp2)):
                    for m in VAL_ACT:
                        nc.scalar.activation(
                            trash_a[:], xc[:], ACT.Relu,
                            bias=bias_f[:, m:m + 1], scale=1.0,
                            accum_out=strip_a[:, _val_col(ch, m):
                                              _val_col(ch, m) + 1])
            with sc(nc, "bin_dve"):
                for ch, xc in ((0, xp1), (1, xp2)):
                    for m in VAL_DVE:
                        nc.vector.tensor_scalar(
                            trash_d[:], xc[:], float(m), 0.0,
                            AluOp.subtract, AluOp.max,
                            accum_out=strip[:, _val_col(ch, m):
                                            _val_col(ch, m) + 1])
                    for m in CNT_DVE:
                        nc.vector.tensor_scalar(
                            trash_d[:], xc[:], float(m), 0.0,
                            AluOp.is_ge, AluOp.add,
                            accum_out=strip[:, _cnt_col(ch, m):
                                            _cnt_col(ch, m) + 1])

            # CNT_HALF: chunk-0 half on DVE, chunk-1 half on ACT Sign
            ih = N_CSPLIT.index(CNT_HALF)
            for ch, xc in ((0, xp1), (1, xp2)):
                c0 = _cnt_col(ch, CNT_HALF)
                c1 = _csplit_col(ch, ih)
                nc.vector.tensor_scalar(
                    trash_d[:, 0:CHUNK], xc[:, 0:CHUNK], float(CNT_HALF), 0.0,
                    AluOp.is_ge, AluOp.add,
                    accum_out=strip[:, c0:c0 + 1])
                nc.scalar.activation(
                    trash_a[:, CHUNK:2 * CHUNK], xc[:, CHUNK:2 * CHUNK],
                    ACT.Sign, bias=bias_f[:, CNT_HALF:CNT_HALF + 1], scale=1.0,
                    accum_out=strip_a[:, c1:c1 + 1])

            nc.sync.dma_start(out[:], strip[:])
            nc.sync.dma_start(out_a[:], strip_a[:])

    nc.compile()
    nc.m = get_hw_module(nc.m)
    return nc


_NC_CACHE = None


def _get_nc():
    global _NC_CACHE
    if _NC_CACHE is None:
        _NC_CACHE = build_nc()
    return _NC_CACHE


def make_in_maps(predictions, targets):
    in_maps = []
    for k in range(N_CORES):
        b = k // CORES_PER_BATCH
        d0 = (k % CORES_PER_BATCH) * D_SH
        pr = np.ascontiguousarray(
            predictions[b, :, d0:d0 + D_SH]).reshape(C, P, F)
        tg = np.ascontiguousarray(
            targets[b, 1:, d0:d0 + D_SH]).reshape(2, P, F)
        in_maps.append({"pred": pr, "targ": tg})
    return in_maps


def decode(strips):
    """strips: list of N_CORES arrays [P, N_COLS] -> final scalar (f64)."""
    n_row_elems = float(P * F)
    n_chunk_elems = float(P * CHUNK)
    Bv = np.zeros((B, 2, NBINS))       # B_m, m = 0..32
    Ng = np.zeros((B, 2, NBINS + 1))   # N_{>=m}, m = 1..33 (33 stays 0)
    Graw = np.zeros((B, 2, NBINS))
    for k in range(N_CORES):
        b = k // CORES_PER_BATCH
        s = strips[k].astype(np.float64).sum(axis=0)   # [N_COLS]
        for ch in range(2):
            for m in range(NBINS):
                Graw[b, ch, m] += s[_val_col(ch, m)]
            for i, m in enumerate(VAL_SPLIT):
                Graw[b, ch, m] += s[_split_col(ch, i)]
            for m in CNT_DVE:
                Ng[b, ch, m - 1] += s[_cnt_col(ch, m)]
            for i, m in enumerate(N_CSPLIT):
                c0 = s[_cnt_col(ch, m)]
                c1 = s[_csplit_col(ch, i)]
                if m in CNT_ACT:
                    Ng[b, ch, m - 1] += (0.5 * (c0 + n_chunk_elems)
                                         + 0.5 * (c1 + n_chunk_elems))
                elif m == CNT_HALF:
                    Ng[b, ch, m - 1] += c0 + 0.5 * (c1 + n_chunk_elems)
                else:
                    Ng[b, ch, m - 1] += c0 + c1
    # G_m = B_m + sum_{i>m} N_{>=i}  ->  B_m = G_m - suffix
    for b in range(B):
        for ch in range(2):
            for m in range(NBINS):
                # sum_{i>m} N_{>=i}: Ng index i-1 over i = m+1..33
                suffix_m = Ng[b, ch, m:NBINS].sum()
                Bv[b, ch, m] = Graw[b, ch, m] - suffix_m
    # P_m = B_m - B_{m+1};  C_m = N_{>=m} - N_{>=m+1}
    Pm = np.concatenate([Bv[:, :, :-1] - Bv[:, :, 1:], Bv[:, :, -1:]], axis=2)
    Cm = Ng[:, :, :NBINS - 1] - Ng[:, :, 1:NBINS]    # m = 1..32

    s_bg = Pm[:, :, 0:1]
    s_i = Pm[:, :, 1:]
    n_i = Cm
    dice = 1.0 - (2.0 * s_i + EPS) / (s_bg + s_i + n_i + EPS)
    present = (n_i > 0.5).astype(np.float64)
    per_class = (dice * present).sum(axis=(0, 2)) / np.maximum(
        present.sum(axis=(0, 2)), 1.0)
    return per_class.mean()


def kernel(predictions, targets):
    predictions = np.asarray(predictions, dtype=np.float32)
    targets = np.asarray(targets, dtype=np.int32)
    nc = _get_nc()
    in_maps = make_in_maps(predictions, targets)
    res = bass_utils.run_bass_kernel_spmd(
        nc, in_maps, core_ids=list(range(N_CORES)))
    strips = [res.results[k]["out"] + res.results[k]["out_a"]
              for k in range(N_CORES)]
    return np.float32(decode(strips))



# revision 18
# speedup vs baseline: 1.2896x; 1.2896x over previous
"""BatchBlobLoss Trainium2 kernel (8-core SPMD), bf16 + TensorE-reduce.

Reference computation:
  p = softmax(predictions, axis=1)[:, 1:]          # foreground class probs
  per (b, c): segment-sum of p keyed by instance id t = targets[b, c]
  soft-dice per (b, c, instance), masked mean -> scalar.

Measured HW facts driving the design (microbench):
  - DVE tensor_scalar WITHOUT accum runs at 4x on packed bf16
    (~2.3us / [128,8192] pass); WITH accum_out it lowers to
    TENSOR_SCALAR_CACHE_REDUCE which is hard-capped at 1x (~8.7us).
  - ACT ACTIVATE is always 1x (~7.1us + 0.3us accumulator read).
  - The TensorEngine is otherwise idle; out[i,j] = sum_part lhsT[part,i]
    * rhs[part,j] with a fixed ones[128,1] stationary reduces a [128,512]
    tile to [1,512] in 512 cycles, accumulating 16 chunks into one PSUM
    row (~3.5us per full [128,8192] reduction, no stationary reloads).

Per core (cores k = 0..7: batch b = k//4, D-slice 16*(k%4)..+16):
  prep: softmax via differences (2 ACT exps, ACT ln+exp reciprocal),
  x = t + p packed in bf16.
  bins: per channel, value bins m=0..32 (G_m = sum relu(x-m), via
  M_m = sum min(x,m), G_m = G_0 - M_m) and count bins m=1..32
  (N_{>=m} = sum [t >= m]) are split:
    PE  : DVE 4x transform (min / is_ge, no accum) -> 16 accumulating
          matmuls vs ones -> one PSUM row of 512 partials
    ACT : Relu / Sign with per-partition bias + accumulator
    DVE : 1x tensor_scalar+accum (filler bins)
  Host (float64) sums partials and evaluates the dice formula.
"""
import numpy as np
import ml_dtypes
from contextlib import ExitStack

import concourse.bass as bass
import concourse.tile as tile
from concourse import bacc, mybir
from concourse import bass_utils
from concourse.bass_interp import get_hw_module

# problem shape (hardcoded per contest rules)
B, C, D, H, W = 2, 3, 64, 256, 256
M = 32
EPS = 1e-5
N_CORES = 8
CORES_PER_BATCH = 4
D_SH = D // CORES_PER_BATCH      # 16 depth slices per core
P = 128
NVOX = D_SH * H * W              # 1,048,576 voxels per core per channel
F = NVOX // P                    # 8192
CHUNK = 4096
NCHUNK = F // CHUNK              # 2
NBINS = 33
MMW = 512                        # matmul moving width (one PSUM bank row)
NMM = F // MMW                   # 16 accumulating matmuls per reduction

# --- per-channel engine assignment (tunable) -------------------------------
VAL_ACT_MS = list(range(1, 10))      # ACT Relu bins -> G_m
VAL_PE_MS = list(range(10, 31))      # DVE min-transform + PE reduce -> M_m
VAL_DVE_MS = [31, 32]                # DVE (min, add) accum -> M_m
CNT_ACT_MS = list(range(1, 10))      # ACT Sign bins -> 2N-n
CNT_PE_MS = list(range(10, 30))      # DVE is_ge-transform + PE reduce -> N
CNT_DVE_MS = [30, 31, 32]            # DVE (is_ge, add) accum -> N

# PE row map: row index in the PSUM bank per (ch, kind, m)
PE_BINS = []
for ch in range(2):
    PE_BINS.append((ch, "g0", 0))          # sum x  (= G_0 = M_33), no transform
    for m in VAL_PE_MS:
        PE_BINS.append((ch, "v", m))
    for m in CNT_PE_MS:
        PE_BINS.append((ch, "c", m))
PE_ROW = {bin_: r for r, bin_ in enumerate(PE_BINS)}
NPE = len(PE_BINS)
assert NPE <= 128

_D_VPC = len(VAL_DVE_MS)
_D_CPC = len(CNT_DVE_MS)
ND = 2 * (_D_VPC + _D_CPC)
_A_VPC = len(VAL_ACT_MS)
_A_CPC = len(CNT_ACT_MS)
NA = 2 * (_A_VPC + _A_CPC)


def _dcol(ch, kind, i):
    base = ch * (_D_VPC + _D_CPC)
    return base + (i if kind == "v" else _D_VPC + i)


def _acol(ch, kind, i):
    if kind == "c":
        return ch * _A_CPC + i
    return 2 * _A_CPC + ch * _A_VPC + i


F32 = mybir.dt.float32
BF16 = mybir.dt.bfloat16


def build_nc(scopes=False):
    AluOp = mybir.AluOpType
    ACT = mybir.ActivationFunctionType

    import contextlib

    def sc(nc, name):
        return nc.named_scope(name) if scopes else contextlib.nullcontext()

    nc = bacc.Bacc("TRN2", target_bir_lowering=False, debug=False,
                   num_devices=N_CORES)
    pred = nc.dram_tensor("pred", [C, P, F], BF16, kind="ExternalInput").ap()
    targ = nc.dram_tensor("targ", [2, P, F], BF16, kind="ExternalInput").ap()
    out_d = nc.dram_tensor("out_d", [P, ND], F32, kind="ExternalOutput").ap()
    out_a = nc.dram_tensor("out_a", [P, NA], F32, kind="ExternalOutput").ap()
    out_pe = nc.dram_tensor("out_pe", [NPE, MMW], F32,
                            kind="ExternalOutput").ap()

    with tile.TileContext(nc) as tc:
        with ExitStack() as ctx:
            pool = ctx.enter_context(tc.tile_pool(name="main", bufs=1))
            ppool = ctx.enter_context(tc.psum_pool(name="pe", bufs=1))

            bias_i = pool.tile([P, NBINS], mybir.dt.int32, tag="bias_i")
            nc.gpsimd.iota(bias_i[:], [[1, NBINS]], channel_multiplier=0)
            bias_f = pool.tile([P, NBINS], F32, tag="bias_f")
            nc.vector.tensor_scalar(bias_f[:], bias_i[:], -1.0, None,
                                    AluOp.mult)
            bias_h = pool.tile([P, NBINS], F32, tag="bias_h")
            nc.vector.tensor_scalar(bias_h[:], bias_f[:], 0.5, None,
                                    AluOp.add)

            strip_d = pool.tile([P, ND], F32, tag="strip_d")
            strip_a = pool.tile([P, NA], F32, tag="strip_a")
            nc.vector.memset(strip_d[:], 0.0)
            nc.scalar.memzero(strip_a[:])

            ones = pool.tile([P, 1], BF16, tag="ones")
            nc.vector.memset(ones[:], 1.0)

            t1 = pool.tile([P, F], BF16, tag="t1")
            t2 = pool.tile([P, F], BF16, tag="t2")
            x1 = pool.tile([P, F], BF16, tag="x1")
            x2 = pool.tile([P, F], BF16, tag="x2")

            trash_d = pool.tile([P, F], BF16, tag="trash_d")
            trash_a = pool.tile([P, F], BF16, tag="trash_a")

            ltiles = []
            for chunk in range(NCHUNK):
                sl = bass.ts(chunk, CHUNK)
                l0 = pool.tile([P, CHUNK], BF16, tag="l0", bufs=NCHUNK)
                l1 = pool.tile([P, CHUNK], BF16, tag="l1", bufs=NCHUNK)
                l2 = pool.tile([P, CHUNK], BF16, tag="l2", bufs=NCHUNK)
                ltiles.append((l0, l1, l2))
                nc.sync.dma_start(t1[:, sl], targ[0, :, sl])
                nc.sync.dma_start(l0[:], pred[0, :, sl])
                nc.sync.dma_start(l1[:], pred[1, :, sl])
                nc.sync.dma_start(l2[:], pred[2, :, sl])
                nc.sync.dma_start(t2[:, sl], targ[1, :, sl])

                with sc(nc, f"prep_sub{chunk}"):
                    nc.vector.tensor_tensor(l0[:], l0[:], l1[:],
                                            AluOp.subtract)
                    nc.vector.tensor_tensor(l2[:], l2[:], l1[:],
                                            AluOp.subtract)
                with sc(nc, f"prep_exp{chunk}"):
                    nc.scalar.activation(l0[:], l0[:], ACT.Exp)
                    nc.scalar.activation(l2[:], l2[:], ACT.Exp)
                with sc(nc, f"prep_sum{chunk}"):
                    nc.vector.tensor_tensor(l1[:], l0[:], l2[:], AluOp.add)
                    nc.vector.tensor_scalar(l1[:], l1[:], 1.0, None,
                                            AluOp.add)

            for chunk in range(NCHUNK):
                sl = bass.ts(chunk, CHUNK)
                l0, l1, l2 = ltiles[chunk]
                with sc(nc, f"prep_recip{chunk}"):
                    # r = 1/s = exp(-ln(s)); Exp+Ln share one ACT table set
                    nc.scalar.activation(l1[:], l1[:], ACT.Ln)
                    nc.scalar.activation(l1[:], l1[:], ACT.Exp, scale=-1.0)
                with sc(nc, f"prep_pack{chunk}"):
                    nc.vector.tensor_tensor(l2[:], l2[:], l1[:], AluOp.mult)
                    nc.vector.tensor_tensor(x1[:, sl], t1[:, sl], l1[:],
                                            AluOp.add)
                    nc.vector.tensor_tensor(x2[:, sl], t2[:, sl], l2[:],
                                            AluOp.add)

            xs = (x1, x2)
            ts = (t1, t2)

            # PE reduction slots: each PSUM bank holds 4 bins (the 4 legal
            # tile positions 0/32/64/96); a single DVE full-bank copy then
            # stages 4 bins at once and SP DMAs the rows out (DMA cannot
            # read PSUM directly).
            QPOS = (0, 32, 64, 96)
            pe_state = {"pslot": None, "quad": []}

            def pe_flush():
                pslot = pe_state["pslot"]
                if pslot is None:
                    return
                stage = pool.tile([P, MMW], F32, tag="stage", bufs=2,
                                  name="stage")
                nc.vector.tensor_copy(stage[:], pslot[:])
                for row, pos in pe_state["quad"]:
                    nc.sync.dma_start(out_pe[row:row + 1, :],
                                      stage[pos:pos + 1, :])
                pe_state["pslot"] = None
                pe_state["quad"] = []

            def pe_reduce(row, src):
                """16 accumulating matmuls reduce src into one PSUM row."""
                if pe_state["pslot"] is None:
                    pe_state["pslot"] = ppool.tile([P, MMW], F32,
                                                   tag="pslot", bufs=8,
                                                   name="pslot")
                pslot = pe_state["pslot"]
                pos = QPOS[len(pe_state["quad"])]
                for k in range(NMM):
                    nc.tensor.matmul(
                        pslot[pos:pos + 1, :], ones[:],
                        src[:, k * MMW:(k + 1) * MMW],
                        start=(k == 0), stop=(k == NMM - 1),
                        tile_position=(0, pos))
                pe_state["quad"].append((row, pos))
                if len(pe_state["quad"]) == 4:
                    pe_flush()

            # ACT count bins first (t is ready long before x)
            with sc(nc, "act_cnt"):
                for ch in range(2):
                    for i, m in enumerate(CNT_ACT_MS):
                        col = _acol(ch, "c", i)
                        nc.scalar.activation(
                            trash_a[:], ts[ch][:], ACT.Sign,
                            bias=bias_h[:, m:m + 1], scale=1.0,
                            accum_out=strip_a[:, col:col + 1])
            with sc(nc, "act_val"):
                for ch in range(2):
                    for i, m in enumerate(VAL_ACT_MS):
                        col = _acol(ch, "v", i)
                        nc.scalar.activation(
                            trash_a[:], xs[ch][:], ACT.Relu,
                            bias=bias_f[:, m:m + 1], scale=1.0,
                            accum_out=strip_a[:, col:col + 1])

            # PE G0 bins: reduce x directly (no DVE transform needed)
            with sc(nc, "pe_g0"):
                for ch in range(2):
                    pe_reduce(PE_ROW[(ch, "g0", 0)], xs[ch][:])

            # PE transform+reduce bins, with DVE filler bins woven in so the
            # DVE stays busy while PE drains the transform queue
            dve_solo = []
            for ch in range(2):
                for i, m in enumerate(VAL_DVE_MS):
                    dve_solo.append((ch, "v", i, m))
                for i, m in enumerate(CNT_DVE_MS):
                    dve_solo.append((ch, "c", i, m))

            pe_stream = [b for b in PE_BINS if b[1] != "g0"]
            n_pe = len(pe_stream)
            solo_every = max(1, n_pe // max(len(dve_solo), 1))
            solo_iter = iter(dve_solo)

            with sc(nc, "bins"):
                for j, (ch, kind, m) in enumerate(pe_stream):
                    trash_pe = pool.tile([P, F], BF16, tag="trash_pe",
                                         bufs=3, name="trash_pe")
                    if kind == "v":
                        nc.vector.tensor_scalar(
                            trash_pe[:], xs[ch][:], float(m), None,
                            AluOp.min)
                    else:
                        nc.vector.tensor_scalar(
                            trash_pe[:], ts[ch][:], float(m) - 0.5, None,
                            AluOp.is_ge)
                    pe_reduce(PE_ROW[(ch, kind, m)], trash_pe[:])
                    if (j + 1) % solo_every == 0:
                        try:
                            sch, skind, si, sm = next(solo_iter)
                        except StopIteration:
                            continue
                        col = _dcol(sch, skind, si)
                        if skind == "v":
                            nc.vector.tensor_scalar(
                                trash_d[:], xs[sch][:], float(sm), 0.0,
                                AluOp.min, AluOp.add,
                                accum_out=strip_d[:, col:col + 1])
                        else:
                            nc.vector.tensor_scalar(
                                trash_d[:], ts[sch][:], float(sm) - 0.5, 0.0,
                                AluOp.is_ge, AluOp.add,
                                accum_out=strip_d[:, col:col + 1])
                for rest in solo_iter:
                    sch, skind, si, sm = rest
                    col = _dcol(sch, skind, si)
                    if skind == "v":
                        nc.vector.tensor_scalar(
                            trash_d[:], xs[sch][:], float(sm), 0.0,
                            AluOp.min, AluOp.add,
                            accum_out=strip_d[:, col:col + 1])
                    else:
                        nc.vector.tensor_scalar(
                            trash_d[:], ts[sch][:], float(sm) - 0.5, 0.0,
                            AluOp.is_ge, AluOp.add,
                            accum_out=strip_d[:, col:col + 1])

            pe_flush()
            nc.sync.dma_start(out_d[:], strip_d[:])
            nc.sync.dma_start(out_a[:], strip_a[:])

    nc.compile()
    nc.m = get_hw_module(nc.m)
    return nc


_NC_CACHE = None


def _get_nc():
    global _NC_CACHE
    if _NC_CACHE is None:
        _NC_CACHE = build_nc()
    return _NC_CACHE


def make_in_maps(predictions, targets):
    bf16 = ml_dtypes.bfloat16
    in_maps = []
    for k in range(N_CORES):
        b = k // CORES_PER_BATCH
        d0 = (k % CORES_PER_BATCH) * D_SH
        pr = np.ascontiguousarray(
            predictions[b, :, d0:d0 + D_SH]).reshape(C, P, F).astype(bf16)
        tg = np.ascontiguousarray(
            targets[b, 1:, d0:d0 + D_SH]).reshape(2, P, F).astype(bf16)
        in_maps.append({"pred": pr, "targ": tg})
    return in_maps


def decode(strips):
    """strips: list of N_CORES dicts with out_d/out_a/out_pe -> scalar."""
    n_core_elems = float(P * F)
    G = np.zeros((B, 2, NBINS))        # G_m = sum relu(x - m)
    Mv = np.zeros((B, 2, NBINS))       # M_m = sum min(x, m)
    G0 = np.zeros((B, 2))              # sum x
    Ng = np.zeros((B, 2, NBINS + 1))   # N_{>=m}; index m = 1..32
    for k in range(N_CORES):
        b = k // CORES_PER_BATCH
        sd = strips[k]["out_d"].astype(np.float64).sum(axis=0)
        sa = strips[k]["out_a"].astype(np.float64).sum(axis=0)
        spe = strips[k]["out_pe"].astype(np.float64).sum(axis=1)
        for ch in range(2):
            G0[b, ch] += spe[PE_ROW[(ch, "g0", 0)]]
            for m in VAL_PE_MS:
                Mv[b, ch, m] += spe[PE_ROW[(ch, "v", m)]]
            for m in CNT_PE_MS:
                Ng[b, ch, m] += spe[PE_ROW[(ch, "c", m)]]
            for i, m in enumerate(VAL_DVE_MS):
                Mv[b, ch, m] += sd[_dcol(ch, "v", i)]
            for i, m in enumerate(CNT_DVE_MS):
                Ng[b, ch, m] += sd[_dcol(ch, "c", i)]
            for i, m in enumerate(VAL_ACT_MS):
                G[b, ch, m] += sa[_acol(ch, "v", i)]
            for i, m in enumerate(CNT_ACT_MS):
                # sum of sign(t - m + .5) = 2*N_{>=m} - n
                Ng[b, ch, m] += 0.5 * (sa[_acol(ch, "c", i)] + n_core_elems)
    # value bins: G_0 = sum x; min-family bins -> G_m = G_0 - M_m
    G[:, :, 0] = G0
    for m in VAL_PE_MS + VAL_DVE_MS:
        G[:, :, m] = G0 - Mv[:, :, m]
    # B_m = G_m - sum_{i>m} N_{>=i}
    Bv = np.zeros((B, 2, NBINS))
    for b in range(B):
        for ch in range(2):
            for m in range(NBINS):
                Bv[b, ch, m] = G[b, ch, m] - Ng[b, ch, m + 1:NBINS].sum()
    # P_m = B_m - B_{m+1};  C_m = N_{>=m} - N_{>=m+1}
    Pm = np.concatenate([Bv[:, :, :-1] - Bv[:, :, 1:], Bv[:, :, -1:]], axis=2)
    Cm = Ng[:, :, 1:NBINS] - Ng[:, :, 2:NBINS + 1]

    s_bg = Pm[:, :, 0:1]
    s_i = Pm[:, :, 1:]
    n_i = Cm
    dice = 1.0 - (2.0 * s_i + EPS) / (s_bg + s_i + n_i + EPS)
    present = (n_i > 0.5).astype(np.float64)
    per_class = (dice * present).sum(axis=(0, 2)) / np.maximum(
        present.sum(axis=(0, 2)), 1.0)
    return per_class.mean()


def kernel(predictions, targets):
    predictions = np.asarray(predictions, dtype=np.float32)
    targets = np.asarray(targets, dtype=np.int32)
    nc = _get_nc()
    in_maps = make_in_maps(predictions, targets)
    res = bass_utils.run_bass_kernel_spmd(
        nc, in_maps, core_ids=list(range(N_CORES)))
    strips = [{n: res.results[k][n] for n in ("out_d", "out_a", "out_pe")}
              for k in range(N_CORES)]
    return np.float32(decode(strips))


# revision 20
# speedup vs baseline: 1.6534x; 1.2821x over previous
"""BatchBlobLoss Trainium2 kernel (8-core SPMD), bf16 + TensorE-reduce.

Reference computation:
  p = softmax(predictions, axis=1)[:, 1:]          # foreground class probs
  per (b, c): segment-sum of p keyed by instance id t = targets[b, c]
  soft-dice per (b, c, instance), masked mean -> scalar.

Measured HW facts driving the design (microbench):
  - DVE tensor_scalar WITHOUT accum runs at 4x on packed bf16
    (~2.3us / [128,8192] pass); WITH accum_out it lowers to
    TENSOR_SCALAR_CACHE_REDUCE which is hard-capped at 1x (~8.7us).
  - ACT ACTIVATE is always 1x (~7.1us + 0.3us accumulator read).
  - The TensorEngine is otherwise idle; out[i,j] = sum_part lhsT[part,i]
    * rhs[part,j] with a fixed ones[128,1] stationary reduces a [128,512]
    tile to [1,512] in 512 cycles, accumulating 16 chunks into one PSUM
    row (~3.5us per full [128,8192] reduction, no stationary reloads).

Per core (cores k = 0..7: batch b = k//4, D-slice 16*(k%4)..+16):
  prep: softmax via differences (2 ACT exps, ACT ln+exp reciprocal),
  x = t + p packed in bf16.
  bins: per channel, value bins m=0..32 (G_m = sum relu(x-m), via
  M_m = sum min(x,m), G_m = G_0 - M_m) and count bins m=1..32
  (N_{>=m} = sum [t >= m]) are split:
    PE  : DVE 4x transform (min / is_ge, no accum) -> 16 accumulating
          matmuls vs ones -> one PSUM row of 512 partials
    ACT : Relu / Sign with per-partition bias + accumulator
    DVE : 1x tensor_scalar+accum (filler bins)
  Host (float64) sums partials and evaluates the dice formula.
"""
import numpy as np
import ml_dtypes
from contextlib import ExitStack

import concourse.bass as bass
import concourse.tile as tile
from concourse import bacc, mybir
from concourse import bass_utils
from concourse.bass_interp import get_hw_module

# problem shape (hardcoded per contest rules)
B, C, D, H, W = 2, 3, 64, 256, 256
M = 32
EPS = 1e-5
N_CORES = 8
CORES_PER_BATCH = 4
D_SH = D // CORES_PER_BATCH      # 16 depth slices per core
P = 128
NVOX = D_SH * H * W              # 1,048,576 voxels per core per channel
F = NVOX // P                    # 8192
CHUNK = 2048
NCHUNK = F // CHUNK              # 4
NBINS = 33
MMW = 512                        # matmul moving width (one PSUM bank row)
NMM = F // MMW                   # 16 accumulating matmuls per reduction

# --- per-channel engine assignment (tunable) -------------------------------
VAL_ACT_MS = list(range(1, 10))      # ACT Relu bins -> G_m
VAL_PE_MS = list(range(10, 31))      # DVE min-transform + PE reduce -> M_m
VAL_DVE_MS = [31, 32]                # DVE (min, add) accum -> M_m
CNT_ACT_MS = list(range(1, 11))      # ACT Sign bins -> 2N-n
CNT_PE_MS = list(range(11, 30))      # DVE is_ge-transform + PE reduce -> N
CNT_DVE_MS = [30, 31, 32]            # DVE (is_ge, add) accum -> N

# PE row map: row index in the PSUM bank per (ch, kind, m)
PE_BINS = []
for ch in range(2):
    PE_BINS.append((ch, "g0", 0))          # sum x  (= G_0 = M_33), no transform
    for m in CNT_PE_MS:
        PE_BINS.append((ch, "c", m))
    for m in VAL_PE_MS:
        PE_BINS.append((ch, "v", m))
PE_ROW = {bin_: r for r, bin_ in enumerate(PE_BINS)}
NPE = len(PE_BINS)
assert NPE <= 128

_D_VPC = len(VAL_DVE_MS)
_D_CPC = len(CNT_DVE_MS)
ND = 2 * (_D_VPC + _D_CPC)
_A_VPC = len(VAL_ACT_MS)
_A_CPC = len(CNT_ACT_MS)
NA = 2 * (_A_VPC + _A_CPC)


def _dcol(ch, kind, i):
    base = ch * (_D_VPC + _D_CPC)
    return base + (i if kind == "v" else _D_VPC + i)


def _acol(ch, kind, i):
    if kind == "c":
        return ch * _A_CPC + i
    return 2 * _A_CPC + ch * _A_VPC + i


F32 = mybir.dt.float32
BF16 = mybir.dt.bfloat16


def build_nc(scopes=False):
    AluOp = mybir.AluOpType
    ACT = mybir.ActivationFunctionType

    import contextlib

    def sc(nc, name):
        return nc.named_scope(name) if scopes else contextlib.nullcontext()

    nc = bacc.Bacc("TRN2", target_bir_lowering=False, debug=False,
                   num_devices=N_CORES)
    pred = nc.dram_tensor("pred", [C, P, F], BF16, kind="ExternalInput").ap()
    targ = nc.dram_tensor("targ", [2, P, F], BF16, kind="ExternalInput").ap()
    out_d = nc.dram_tensor("out_d", [P, ND], F32, kind="ExternalOutput").ap()
    out_a = nc.dram_tensor("out_a", [P, NA], F32, kind="ExternalOutput").ap()
    out_pe = nc.dram_tensor("out_pe", [NPE, MMW], F32,
                            kind="ExternalOutput").ap()

    with tile.TileContext(nc) as tc:
        with ExitStack() as ctx:
            pool = ctx.enter_context(tc.tile_pool(name="main", bufs=1))
            ppool = ctx.enter_context(tc.psum_pool(name="pe", bufs=1))

            bias_i = pool.tile([P, NBINS], mybir.dt.int32, tag="bias_i")
            nc.gpsimd.iota(bias_i[:], [[1, NBINS]], channel_multiplier=0)
            bias_f = pool.tile([P, NBINS], F32, tag="bias_f")
            nc.vector.tensor_scalar(bias_f[:], bias_i[:], -1.0, None,
                                    AluOp.mult)
            bias_h = pool.tile([P, NBINS], F32, tag="bias_h")
            nc.vector.tensor_scalar(bias_h[:], bias_f[:], 0.5, None,
                                    AluOp.add)

            strip_d = pool.tile([P, ND], F32, tag="strip_d")
            strip_a = pool.tile([P, NA], F32, tag="strip_a")
            nc.vector.memset(strip_d[:], 0.0)
            nc.scalar.memzero(strip_a[:])

            ones = pool.tile([P, 1], BF16, tag="ones")
            nc.vector.memset(ones[:], 1.0)

            t1 = pool.tile([P, F], BF16, tag="t1")
            t2 = pool.tile([P, F], BF16, tag="t2")
            x1 = pool.tile([P, F], BF16, tag="x1")
            x2 = pool.tile([P, F], BF16, tag="x2")

            trash_d = pool.tile([P, F], BF16, tag="trash_d")
            trash_a = pool.tile([P, F], BF16, tag="trash_a")

            ltiles = []
            for chunk in range(NCHUNK):
                sl = bass.ts(chunk, CHUNK)
                l0 = pool.tile([P, CHUNK], BF16, tag="l0", bufs=NCHUNK)
                l1 = pool.tile([P, CHUNK], BF16, tag="l1", bufs=NCHUNK)
                l2 = pool.tile([P, CHUNK], BF16, tag="l2", bufs=NCHUNK)
                ltiles.append((l0, l1, l2))
                nc.sync.dma_start(t1[:, sl], targ[0, :, sl])
                nc.sync.dma_start(l0[:], pred[0, :, sl])
                nc.sync.dma_start(l1[:], pred[1, :, sl])
                nc.sync.dma_start(l2[:], pred[2, :, sl])
                nc.sync.dma_start(t2[:, sl], targ[1, :, sl])

                with sc(nc, f"prep_sub{chunk}"):
                    nc.vector.tensor_tensor(l0[:], l0[:], l1[:],
                                            AluOp.subtract)
                    nc.vector.tensor_tensor(l2[:], l2[:], l1[:],
                                            AluOp.subtract)
                with sc(nc, f"prep_exp{chunk}"):
                    nc.scalar.activation(l0[:], l0[:], ACT.Exp)
                    nc.scalar.activation(l2[:], l2[:], ACT.Exp)
                with sc(nc, f"prep_sum{chunk}"):
                    nc.vector.tensor_tensor(l1[:], l0[:], l2[:], AluOp.add)
                    nc.vector.tensor_scalar(l1[:], l1[:], 1.0, None,
                                            AluOp.add)

            for chunk in range(NCHUNK):
                sl = bass.ts(chunk, CHUNK)
                l0, l1, l2 = ltiles[chunk]
                with sc(nc, f"prep_recip{chunk}"):
                    # r = 1/s = exp(-ln(s)); Exp+Ln share one ACT table set
                    nc.scalar.activation(l1[:], l1[:], ACT.Ln)
                    nc.scalar.activation(l1[:], l1[:], ACT.Exp, scale=-1.0)
                with sc(nc, f"prep_pack{chunk}"):
                    nc.vector.tensor_tensor(l2[:], l2[:], l1[:], AluOp.mult)
                    nc.vector.tensor_tensor(x1[:, sl], t1[:, sl], l1[:],
                                            AluOp.add)
                    nc.vector.tensor_tensor(x2[:, sl], t2[:, sl], l2[:],
                                            AluOp.add)

            xs = (x1, x2)
            ts = (t1, t2)

            # PE reduction slots: each PSUM bank holds 4 bins (the 4 legal
            # tile positions 0/32/64/96); a single DVE full-bank copy then
            # stages 4 bins at once and SP DMAs the rows out (DMA cannot
            # read PSUM directly).
            QPOS = (0, 32, 64, 96)
            pe_state = {"pslot": None, "quad": [], "pending": []}

            def pe_flush_one():
                pslot, quad = pe_state["pending"].pop(0)
                stage = pool.tile([P, MMW], F32, tag="stage", bufs=2,
                                  name="stage")
                nc.vector.tensor_copy(stage[:], pslot[:])
                for row, pos in quad:
                    nc.sync.dma_start(out_pe[row:row + 1, :],
                                      stage[pos:pos + 1, :])

            def pe_flush_all():
                if pe_state["quad"]:
                    pe_state["pending"].append(
                        (pe_state["pslot"], pe_state["quad"]))
                    pe_state["pslot"] = None
                    pe_state["quad"] = []
                while pe_state["pending"]:
                    pe_flush_one()

            def pe_reduce(row, src):
                """16 accumulating matmuls reduce src into one PSUM row.
                Full banks are staged out lazily (2 banks behind) so the
                DVE copy never stalls waiting for the PE to catch up."""
                if pe_state["pslot"] is None:
                    pe_state["pslot"] = ppool.tile([P, MMW], F32,
                                                   tag="pslot", bufs=8,
                                                   name="pslot")
                pslot = pe_state["pslot"]
                pos = QPOS[len(pe_state["quad"])]
                for k in range(NMM):
                    nc.tensor.matmul(
                        pslot[pos:pos + 1, :], ones[:],
                        src[:, k * MMW:(k + 1) * MMW],
                        start=(k == 0), stop=(k == NMM - 1),
                        tile_position=(0, pos))
                pe_state["quad"].append((row, pos))
                if len(pe_state["quad"]) == 4:
                    pe_state["pending"].append(
                        (pe_state["pslot"], pe_state["quad"]))
                    pe_state["pslot"] = None
                    pe_state["quad"] = []
                    if len(pe_state["pending"]) >= 3:
                        pe_flush_one()

            # ACT count bins first (t is ready long before x)
            with sc(nc, "act_cnt"):
                for ch in range(2):
                    for i, m in enumerate(CNT_ACT_MS):
                        col = _acol(ch, "c", i)
                        nc.scalar.activation(
                            trash_a[:], ts[ch][:], ACT.Sign,
                            bias=bias_h[:, m:m + 1], scale=1.0,
                            accum_out=strip_a[:, col:col + 1])
            with sc(nc, "act_val"):
                for ch in range(2):
                    for i, m in enumerate(VAL_ACT_MS):
                        col = _acol(ch, "v", i)
                        nc.scalar.activation(
                            trash_a[:], xs[ch][:], ACT.Relu,
                            bias=bias_f[:, m:m + 1], scale=1.0,
                            accum_out=strip_a[:, col:col + 1])

            # PE G0 bins: reduce x directly (no DVE transform needed)
            with sc(nc, "pe_g0"):
                for ch in range(2):
                    pe_reduce(PE_ROW[(ch, "g0", 0)], xs[ch][:])

            # PE transform+reduce bins, with DVE filler bins woven in so the
            # DVE stays busy while PE drains the transform queue
            dve_solo = []
            for ch in range(2):
                for i, m in enumerate(VAL_DVE_MS):
                    dve_solo.append((ch, "v", i, m))
                for i, m in enumerate(CNT_DVE_MS):
                    dve_solo.append((ch, "c", i, m))

            pe_stream = [b for b in PE_BINS if b[1] != "g0"]
            n_pe = len(pe_stream)
            solo_every = max(1, n_pe // max(len(dve_solo), 1))
            solo_iter = iter(dve_solo)

            with sc(nc, "bins"):
                for j, (ch, kind, m) in enumerate(pe_stream):
                    trash_pe = pool.tile([P, F], BF16, tag="trash_pe",
                                         bufs=3, name="trash_pe")
                    if kind == "v":
                        nc.vector.tensor_scalar(
                            trash_pe[:], xs[ch][:], float(m), None,
                            AluOp.min)
                    else:
                        nc.vector.tensor_scalar(
                            trash_pe[:], ts[ch][:], float(m) - 0.5, None,
                            AluOp.is_ge)
                    pe_reduce(PE_ROW[(ch, kind, m)], trash_pe[:])
                    if (j + 1) % solo_every == 0:
                        try:
                            sch, skind, si, sm = next(solo_iter)
                        except StopIteration:
                            continue
                        col = _dcol(sch, skind, si)
                        if skind == "v":
                            nc.vector.tensor_scalar(
                                trash_d[:], xs[sch][:], float(sm), 0.0,
                                AluOp.min, AluOp.add,
                                accum_out=strip_d[:, col:col + 1])
                        else:
                            nc.vector.tensor_scalar(
                                trash_d[:], ts[sch][:], float(sm) - 0.5, 0.0,
                                AluOp.is_ge, AluOp.add,
                                accum_out=strip_d[:, col:col + 1])
                for rest in solo_iter:
                    sch, skind, si, sm = rest
                    col = _dcol(sch, skind, si)
                    if skind == "v":
                        nc.vector.tensor_scalar(
                            trash_d[:], xs[sch][:], float(sm), 0.0,
                            AluOp.min, AluOp.add,
                            accum_out=strip_d[:, col:col + 1])
                    else:
                        nc.vector.tensor_scalar(
                            trash_d[:], ts[sch][:], float(sm) - 0.5, 0.0,
                            AluOp.is_ge, AluOp.add,
                            accum_out=strip_d[:, col:col + 1])

            pe_flush_all()
            nc.sync.dma_start(out_d[:], strip_d[:])
            nc.sync.dma_start(out_a[:], strip_a[:])

    nc.compile()
    nc.m = get_hw_module(nc.m)
    return nc


_NC_CACHE = None


def _get_nc():
    global _NC_CACHE
    if _NC_CACHE is None:
        _NC_CACHE = build_nc()
    return _NC_CACHE


def make_in_maps(predictions, targets):
    bf16 = ml_dtypes.bfloat16
    in_maps = []
    for k in range(N_CORES):
        b = k // CORES_PER_BATCH
        d0 = (k % CORES_PER_BATCH) * D_SH
        pr = np.ascontiguousarray(
            predictions[b, :, d0:d0 + D_SH]).reshape(C, P, F).astype(bf16)
        tg = np.ascontiguousarray(
            targets[b, 1:, d0:d0 + D_SH]).reshape(2, P, F).astype(bf16)
        in_maps.append({"pred": pr, "targ": tg})
    return in_maps


def decode(strips):
    """strips: list of N_CORES dicts with out_d/out_a/out_pe -> scalar."""
    n_core_elems = float(P * F)
    G = np.zeros((B, 2, NBINS))        # G_m = sum relu(x - m)
    Mv = np.zeros((B, 2, NBINS))       # M_m = sum min(x, m)
    G0 = np.zeros((B, 2))              # sum x
    Ng = np.zeros((B, 2, NBINS + 1))   # N_{>=m}; index m = 1..32
    for k in range(N_CORES):
        b = k // CORES_PER_BATCH
        sd = strips[k]["out_d"].astype(np.float64).sum(axis=0)
        sa = strips[k]["out_a"].astype(np.float64).sum(axis=0)
        spe = strips[k]["out_pe"].astype(np.float64).sum(axis=1)
        for ch in range(2):
            G0[b, ch] += spe[PE_ROW[(ch, "g0", 0)]]
            for m in VAL_PE_MS:
                Mv[b, ch, m] += spe[PE_ROW[(ch, "v", m)]]
            for m in CNT_PE_MS:
                Ng[b, ch, m] += spe[PE_ROW[(ch, "c", m)]]
            for i, m in enumerate(VAL_DVE_MS):
                Mv[b, ch, m] += sd[_dcol(ch, "v", i)]
            for i, m in enumerate(CNT_DVE_MS):
                Ng[b, ch, m] += sd[_dcol(ch, "c", i)]
            for i, m in enumerate(VAL_ACT_MS):
                G[b, ch, m] += sa[_acol(ch, "v", i)]
            for i, m in enumerate(CNT_ACT_MS):
                # sum of sign(t - m + .5) = 2*N_{>=m} - n
                Ng[b, ch, m] += 0.5 * (sa[_acol(ch, "c", i)] + n_core_elems)
    # value bins: G_0 = sum x; min-family bins -> G_m = G_0 - M_m
    G[:, :, 0] = G0
    for m in VAL_PE_MS + VAL_DVE_MS:
        G[:, :, m] = G0 - Mv[:, :, m]
    # B_m = G_m - sum_{i>m} N_{>=i}
    Bv = np.zeros((B, 2, NBINS))
    for b in range(B):
        for ch in range(2):
            for m in range(NBINS):
                Bv[b, ch, m] = G[b, ch, m] - Ng[b, ch, m + 1:NBINS].sum()
    # P_m = B_m - B_{m+1};  C_m = N_{>=m} - N_{>=m+1}
    Pm = np.concatenate([Bv[:, :, :-1] - Bv[:, :, 1:], Bv[:, :, -1:]], axis=2)
    Cm = Ng[:, :, 1:NBINS] - Ng[:, :, 2:NBINS + 1]

    s_bg = Pm[:, :, 0:1]
    s_i = Pm[:, :, 1:]
    n_i = Cm
    dice = 1.0 - (2.0 * s_i + EPS) / (s_bg + s_i + n_i + EPS)
    present = (n_i > 0.5).astype(np.float64)
    per_class = (dice * present).sum(axis=(0, 2)) / np.maximum(
        present.sum(axis=(0, 2)), 1.0)
    return per_class.mean()


def kernel(predictions, targets):
    predictions = np.asarray(predictions, dtype=np.float32)
    targets = np.asarray(targets, dtype=np.int32)
    nc = _get_nc()
    in_maps = make_in_maps(predictions, targets)
    res = bass_utils.run_bass_kernel_spmd(
        nc, in_maps, core_ids=list(range(N_CORES)))
    strips = [{n: res.results[k][n] for n in ("out_d", "out_a", "out_pe")}
              for k in range(N_CORES)]
    return np.float32(decode(strips))
